# revision 6
# baseline (speedup 1.0000x reference)
"""KeyedGRU Trainium2 Bass kernel.

Strategy: data-parallel over batch B=64 across 8 cores (B=8 each), weights
replicated. Per core:
  Phase 0: 16-step key-gate GRU scan (KB=4) -> per-step gates g[16, H].
  Phase 1: 2048-step main GRU. The input-side matmul gi = x @ W_ih.T + bias
  is precomputed in 32-step chunks on the tensor engine (independent of h)
  and interleaved into the per-step idle windows; the sequential per-step
  work is gh = h @ W_hh.T (12 small matmuls, H-on-partitions layout),
  one sigmoid pass (r,i), the n-gate chain on DVE/ACT, and the lerp.
  tanh(z) is computed as 2*sigmoid(2z)-1 so the ACT engine never switches
  activation-table sets between Sigmoid and Tanh.

I/O path (the axon tunnel moves ~50-80 MB/s, so transfer bytes and host
reshapes dominate wall time):
  - x ships as f16 in its natural [BC, T, I] layout (host does a parallel
    dtype cast only, no transpose); the kernel XBAR-transposes the whole
    per-core x into a resident SBUF tile [128, 2, B*T] at startup, and the
    gi matmuls consume strided views of it in bf16 directly.
  - the output ships as int8 in host-natural [T, BC, H] layout: |h| <= 1
    always (h is a convex combination of tanh outputs starting from 0), so
    a fixed 1/127 scale with exact rint (magic-constant rounding) keeps the
    quantization error at 1/254 of absmax. The [128(h), t] -> [128(t), h]
    flip runs on the idle tensor engine (16 PE transposes per 128 steps).
  - the PJRT executable is compiled once per T and cached; the donated
    output buffer for call N+1 is call N's device-resident output, so no
    zero-buffer ships through the tunnel.
"""
import concurrent.futures as _cf

import ml_dtypes
import numpy as np

import concourse.bass as bass
import concourse.tile as tile
from concourse import mybir
from concourse.masks import make_identity

f32 = mybir.dt.float32
f16 = mybir.dt.float16
i8 = mybir.dt.int8
AF = mybir.ActivationFunctionType
ALU = mybir.AluOpType
F16 = np.float16

B, T_FULL, I, H = 64, 2048, 256, 256
KB, KL = 4, 16
NCORE = 8
BC = B // NCORE          # batch per core
M3 = 3 * H               # 768 gate outputs
CH = 32                  # gi chunk (steps)
OCH = 128                # output chunk (steps)
OSCALE = 127.0           # int8 output quantization scale
RMAGIC = 12582912.0      # 1.5 * 2^23: f32 add snaps mantissa to integer


def _fix_waits(nc, limit=1):
    """walrus TPB_CTRL encodes only one sync-wait; split extras onto nops."""
    for func in nc.m.functions:
        for bb in func.blocks:
            out = []
            for ins in bb.instructions:
                si = ins.sync_info
                if si and len(si.on_wait) > limit:
                    waits = list(si.on_wait)
                    for j, w in enumerate(waits[:-limit]):
                        nop = mybir.InstNoOp(name=f"{ins.name}-wfix{j}", ins=[], outs=[])
                        nop.engine = ins.engine
                        nop.sync_info = mybir.SyncInfo(on_wait=[w], on_update=[])
                        out.append(nop)
                    ins.sync_info = mybir.SyncInfo(
                        on_wait=list(waits[-limit:]), on_update=list(si.on_update)
                    )
                out.append(ins)
            bb.instructions = out


def _build(T):
    NCH = T // CH
    nc = bass.Bass("TRN2", num_devices=NCORE)
    x_in = nc.declare_dram_parameter("x", [BC, T, I], f16, isOutput=False)
    wih_d = nc.declare_dram_parameter("wih", [2, 128, M3], f16, isOutput=False)
    whh_d = nc.declare_dram_parameter("whh", [2, 128, M3], f16, isOutput=False)
    brow_d = nc.declare_dram_parameter("brow", [1, M3], f32, isOutput=False)
    bhn_d = nc.declare_dram_parameter("bhn", [2, 128, BC], f32, isOutput=False)
    wmk_d = nc.declare_dram_parameter("wmk", [2, 128, KL * KB], f32, isOutput=False)
    out_d = nc.declare_dram_parameter("out", [T, BC, 2, 128], i8, isOutput=True)

    with tile.TileContext(nc) as tc:
        with (
            tc.tile_pool(name="const", bufs=1) as const,
            tc.tile_pool(name="gips", bufs=2, space="PSUM") as gips,
            tc.tile_pool(name="ghps", bufs=1, space="PSUM") as ghps,
            tc.tile_pool(name="tpps", bufs=1, space="PSUM") as tpps,
            tc.tile_pool(name="gisb", bufs=2) as gisb,
            tc.tile_pool(name="outb", bufs=2) as outb,
            tc.tile_pool(name="qb", bufs=2) as qbp,
            tc.tile_pool(name="tmp", bufs=3) as tmp,
        ):
            # ---- constants ----
            wih_bf = const.tile([128, 2, M3], f16)
            whh_bf = const.tile([128, 2, M3], f16)
            for k in range(2):
                nc.sync.dma_start(out=wih_bf[:, k, :], in_=wih_d[k])
                nc.sync.dma_start(out=whh_bf[:, k, :], in_=whh_d[k])
            wih_sb = const.tile([128, 2, M3], f32)
            whh_sb = const.tile([128, 2, M3], f32)
            nc.vector.tensor_copy(wih_sb, wih_bf)
            nc.vector.tensor_copy(whh_sb, whh_bf)
            brow_sb = const.tile([1, M3], f32)
            nc.sync.dma_start(out=brow_sb, in_=brow_d[:, :])
            bhn_sb = const.tile([128, 2, BC], f32)
            for k in range(2):
                nc.sync.dma_start(out=bhn_sb[:, k, :], in_=bhn_d[k])
            kx_sb = const.tile([128, 2, KL * KB], f32)
            for k in range(2):
                nc.sync.dma_start(out=kx_sb[:, k, :], in_=wmk_d[k])
            ident = const.tile([128, 128], f32)
            make_identity(nc, ident)
            # whole per-core x, XBAR-transposed to put I%128 on partitions:
            # xall[p, k, b*T + t] = x[b, t, k*128+p]
            xall = const.tile([128, 2, BC * T], f16)
            x2d = x_in.rearrange("b t i -> (b t) i")
            for k in range(2):
                nc.sync.dma_start_transpose(
                    out=xall[:, k, :], in_=x2d[:, k * 128 : (k + 1) * 128]
                )
            xv = xall.rearrange("p k (b t) -> p k b t", b=BC)
            ones_sb = const.tile([1, CH * BC], f32)
            nc.vector.memset(ones_sb, 1.0)
            rbuf = const.tile([128, 2, KL, KB], f32)   # reset gates, key scan
            gr_sb = const.tile([128, 2, KL], f32)
            g_sb = const.tile([128, 2, KL], f32)
            h0 = const.tile([128, 2, BC], f32)
            nc.vector.memset(h0, 0.0)
            kgi_sb = const.tile([128, 6, KL * KB], f32)

            def mm(out_ap, lhsT, rhs, start, stop):
                nc.tensor.matmul(out_ap, lhsT, rhs, start=start, stop=stop)

            # ---- phase 0: key-gate scan (KB=4, KL=16) ----
            kgi_ps = gips.tile([128, 6, KL * KB], f32, tag="gi")
            for m in range(6):
                sl = slice(m * 128, (m + 1) * 128)
                mm(kgi_ps[:, m, :], wih_sb[:, 0, sl], kx_sb[:, 0, :], True, False)
                mm(kgi_ps[:, m, :], wih_sb[:, 1, sl], kx_sb[:, 1, :], False, False)
                mm(kgi_ps[:, m, :], brow_sb[:, sl], ones_sb[:, : KL * KB], False, True)
            nc.vector.tensor_copy(kgi_sb, kgi_ps)

            kh = tmp.tile([128, 2, KB], f32, tag="kh")
            nc.vector.memset(kh, 0.0)
            for t in range(KL):
                ksl = slice(t * KB, (t + 1) * KB)
                kgh = ghps.tile([128, 6, KB], f32, tag="gh")
                for m in range(6):
                    sl = slice(m * 128, (m + 1) * 128)
                    mm(kgh[:, m, :], whh_sb[:, 0, sl], kh[:, 0, :], True, False)
                    mm(kgh[:, m, :], whh_sb[:, 1, sl], kh[:, 1, :], False, True)
                sri = tmp.tile([128, 4, KB], f32, tag="sri")
                nc.vector.tensor_add(sri, kgh[:, 0:4, :], kgi_sb[:, 0:4, ksl])
                sig = tmp.tile([128, 4, KB], f32, tag="sig")
                nc.scalar.activation(sig, sri, AF.Sigmoid)
                nc.vector.tensor_copy(rbuf[:, :, t, :], sig[:, 0:2, :])
                t1 = tmp.tile([128, 2, KB], f32, tag="t1")
                nc.vector.tensor_add(t1, kgh[:, 4:6, :], bhn_sb[:, :, 0:KB])
                t2 = tmp.tile([128, 2, KB], f32, tag="t2")
                nc.vector.tensor_mul(t2, t1, sig[:, 0:2, :])
                t3 = tmp.tile([128, 2, KB], f32, tag="t3")
                nc.vector.tensor_add(t3, t2, kgi_sb[:, 4:6, ksl])
                ss = tmp.tile([128, 2, KB], f32, tag="ss")
                nc.scalar.activation(ss, t3, AF.Sigmoid, scale=2.0)
                nn = tmp.tile([128, 2, KB], f32, tag="nn")
                nc.vector.tensor_scalar(nn, ss, 2.0, -1.0, op0=ALU.mult, op1=ALU.add)
                dd = tmp.tile([128, 2, KB], f32, tag="dd")
                nc.vector.tensor_sub(dd, kh, nn)
                ee = tmp.tile([128, 2, KB], f32, tag="ee")
                nc.vector.tensor_mul(ee, dd, sig[:, 2:4, :])
                kh2 = tmp.tile([128, 2, KB], f32, tag="kh")
                nc.vector.tensor_add(kh2, ee, nn)
                kh = kh2
            nc.vector.tensor_reduce(gr_sb, rbuf, axis=mybir.AxisListType.X, op=ALU.add)
            nc.vector.tensor_scalar_mul(g_sb, gr_sb, 1.0 / KB)

            # ---- phase 1: main recurrence ----
            gi_ps_t, gi_sb_t = {}, {}
            pending = []  # deferred GI emission ops: ("mm", c, m, kk) | ("cp", c)

            def queue_gi(c):
                # gi chunk laid out [128, 6, BC, CH] (batch-major free dims,
                # matching xall's (b, t) order)
                gi_ps_t[c] = gips.tile([128, 6, BC, CH], f32, tag="gi", name=f"gi_ps{c}")
                gi_sb_t[c] = gisb.tile([128, 6, BC, CH], f32, tag="gis", name=f"gi_sb{c}")
                for m in range(6):
                    for kk in range(3):
                        pending.append(("mm", c, m, kk))
                pending.append(("cp", c))

            def emit_gi_op(op):
                _, c, m, kk = op if op[0] == "mm" else (None, op[1], None, None)
                if op[0] == "mm":
                    sl = slice(m * 128, (m + 1) * 128)
                    csl = slice(c * CH, (c + 1) * CH)
                    tgt = gi_ps_t[c][:, m, :, :]
                    if kk < 2:
                        mm(tgt, wih_bf[:, kk, sl], xv[:, kk, :, csl], kk == 0, False)
                    else:
                        mm(tgt, brow_sb[:, sl], ones_sb, False, True)
                else:
                    nc.vector.tensor_copy(gi_sb_t[c], gi_ps_t[c])

            # chunk 0 fully up-front; chunk 1 queued so it fills phase-0/early gaps
            queue_gi(0)
            while pending:
                emit_gi_op(pending.pop(0))
            if NCH > 1:
                queue_gi(1)

            hcur = lambda k: h0[:, k, :]      # per-Htile matmul rhs view
            hfull = h0[:, :, :]               # full [128, 2, BC] view for DVE
            ob = None
            for t in range(T):
                c, o = divmod(t, CH)
                ot = t % OCH
                if t % OCH == 0:
                    ob = outb.tile([128, 2, BC, OCH], f32, tag="ob")
                if t % CH == 0 and c + 2 < NCH:
                    queue_gi(c + 2)
                gh = ghps.tile([128, 6, BC], f32, tag="gh")
                for m in range(6):
                    sl = slice(m * 128, (m + 1) * 128)
                    mm(gh[:, m, :], whh_sb[:, 0, sl], hcur(0), True, False)
                    mm(gh[:, m, :], whh_sb[:, 1, sl], hcur(1), False, True)
                # fill PE idle windows with next chunk's gi work
                for _ in range(2):
                    if pending:
                        emit_gi_op(pending.pop(0))
                gsb = gi_sb_t[c]
                sri = tmp.tile([128, 4, BC], f32, tag="sri")
                nc.vector.tensor_add(sri, gh[:, 0:4, :], gsb[:, 0:4, :, o])
                sig = tmp.tile([128, 4, BC], f32, tag="sig")
                nc.scalar.activation(sig, sri, AF.Sigmoid)
                t1 = tmp.tile([128, 2, BC], f32, tag="t1")
                nc.vector.tensor_add(t1, gh[:, 4:6, :], bhn_sb)
                t2 = tmp.tile([128, 2, BC], f32, tag="t2")
                nc.vector.tensor_mul(t2, t1, sig[:, 0:2, :])
                t3 = tmp.tile([128, 2, BC], f32, tag="t3")
                nc.vector.tensor_add(t3, t2, gsb[:, 4:6, :, o])
                # nn = tanh(t3) without leaving the Sigmoid table set
                ss = tmp.tile([128, 2, BC], f32, tag="ss")
                nc.scalar.activation(ss, t3, AF.Sigmoid, scale=2.0)
                nn = tmp.tile([128, 2, BC], f32, tag="nn")
                nc.vector.tensor_scalar(nn, ss, 2.0, -1.0, op0=ALU.mult, op1=ALU.add)
                dd = tmp.tile([128, 2, BC], f32, tag="dd")
                nc.vector.tensor_sub(dd, hfull, nn)
                ee = tmp.tile([128, 2, BC], f32, tag="ee")
                nc.vector.tensor_mul(ee, dd, sig[:, 2:4, :])
                nc.vector.tensor_add(ob[:, :, :, ot], ee, nn)
                if t < KL:
                    hg = tmp.tile([128, 2, BC], f32, tag="hg")
                    for k in range(2):
                        nc.vector.tensor_scalar(
                            hg[:, k, :], ob[:, k, :, ot], g_sb[:, k, t : t + 1],
                            None, op0=ALU.mult,
                        )
                    hcur = (lambda hg_: lambda k: hg_[:, k, :])(hg)
                    hfull = hg[:, :, :]
                else:
                    hcur = (lambda ob_, ot_: lambda k: ob_[:, k, :, ot_])(ob, ot)
                    hfull = ob[:, :, :, ot]
                if ot == OCH - 1:
                    # quantize to int8 in host-natural [t, b, h] layout:
                    # rint via magic constant, PE transpose of each [128h, 128t]
                    # block, then exact integer subtract + int8 cast on DVE.
                    rb = qbp.tile([128, 2, BC, OCH], f32, tag="rb")
                    nc.vector.tensor_scalar(
                        rb, ob, OSCALE, RMAGIC, op0=ALU.mult, op1=ALU.add
                    )
                    obt = qbp.tile([128, BC, 2, 128], i8, tag="obt")
                    for k in range(2):
                        for b in range(BC):
                            tp = tpps.tile([128, 128], f32, tag="tp")
                            nc.tensor.transpose(tp, rb[:, k, b, :], ident)
                            nc.vector.tensor_scalar(
                                obt[:, b, k, :], tp, -RMAGIC, None, op0=ALU.add
                            )
                    nc.sync.dma_start(
                        out=out_d[t - OCH + 1 : t + 1, :, :, :], in_=obt
                    )

    _fix_waits(nc)
    return nc


_RUN = {}


def _runner(T):
    st = _RUN.get(T)
    if st is not None:
        return st
    nc = _build(T)
    st = {"nc": nc, "prev": None}
    from concourse._compat import axon_active

    if axon_active():
        import jax
        import jax.numpy as jnp
        from jax.experimental.shard_map import shard_map
        from jax.sharding import Mesh, NamedSharding, PartitionSpec

        from concourse.bass2jax import (
            _bass_exec_p,
            install_neuronx_cc_hook,
            partition_id_tensor,
        )

        install_neuronx_cc_hook()
        pname = nc.partition_id_tensor.name if nc.partition_id_tensor else None
        in_names, out_names, out_avals = [], [], []
        for alloc in nc.m.functions[0].allocations:
            if not isinstance(alloc, mybir.MemoryLocationSet):
                continue
            name = alloc.memorylocations[0].name
            if alloc.kind == "ExternalInput":
                if name != pname:
                    in_names.append(name)
            elif alloc.kind == "ExternalOutput":
                out_names.append(name)
                out_avals.append(
                    jax.core.ShapedArray(
                        tuple(alloc.tensor_shape), mybir.dt.np(alloc.dtype)
                    )
                )
        n_params = len(in_names)
        n_outs = len(out_avals)
        in_names_all = in_names + out_names + ([pname] if pname else [])
        donate = tuple(range(n_params, n_params + n_outs))

        def _body(*args):
            ops = list(args)
            if pname:
                ops.append(partition_id_tensor())
            return tuple(
                _bass_exec_p.bind(
                    *ops,
                    out_avals=tuple(out_avals),
                    in_names=tuple(in_names_all),
                    out_names=tuple(out_names),
                    lowering_input_output_aliases=(),
                    sim_require_finite=True,
                    sim_require_nnan=True,
                    nc=nc,
                )
            )

        mesh = Mesh(np.asarray(jax.devices()[:NCORE]), ("core",))
        sh = NamedSharding(mesh, PartitionSpec("core"))
        in_specs = (PartitionSpec("core"),) * (n_params + n_outs)
        out_specs = (PartitionSpec("core"),) * n_outs
        sharded = jax.jit(
            shard_map(
                _body, mesh=mesh, in_specs=in_specs, out_specs=out_specs,
                check_rep=False,
            ),
            donate_argnums=donate,
            keep_unused=True,
        )
        oshape = out_avals[0].shape
        zf = jax.jit(
            lambda: jnp.zeros((NCORE * oshape[0],) + oshape[1:], jnp.int8),
            out_shardings=sh,
        )
        st.update(
            axon=True, sharded=sharded, zf=zf, in_names=in_names,
            oshape=oshape, jax=jax,
        )
    else:
        st.update(axon=False)
    _RUN[T] = st
    return st


def _prep_inputs(x, weight_ih, weight_hh, bias_ih, bias_hh, wm_key, T):
    """Host-side packing: per-input arrays concatenated over cores on axis 0.
    x needs only a parallel f32 -> bf16 cast (layout stays [B, T, I])."""
    wih = np.ascontiguousarray(weight_ih.T.reshape(2, 128, M3)).astype(F16)
    whh = np.ascontiguousarray(weight_hh.T.reshape(2, 128, M3)).astype(F16)
    brow = (
        np.asarray(bias_ih, np.float32)
        + np.concatenate(
            [np.asarray(bias_hh[: 2 * H], np.float32), np.zeros(H, np.float32)]
        )
    ).reshape(1, M3)
    bhn = np.ascontiguousarray(
        np.tile(np.asarray(bias_hh[2 * H :], np.float32).reshape(2, 128, 1), (1, 1, BC))
    )
    wmk = np.ascontiguousarray(
        wm_key.transpose(2, 1, 0).reshape(2, 128, KL * KB), np.float32
    )
    xbuf = np.empty((B, T, I), F16)

    def cast_core(c):
        xbuf[c * BC : (c + 1) * BC] = x[c * BC : (c + 1) * BC]

    with _cf.ThreadPoolExecutor(NCORE) as ex:
        list(ex.map(cast_core, range(NCORE)))
    reps = {"wih": wih, "whh": whh, "brow": brow, "bhn": bhn, "wmk": wmk}
    cat = {k: np.concatenate([v] * NCORE, axis=0) for k, v in reps.items()}
    cat["x"] = xbuf
    return cat


def kernel(x, wm_key, weight_ih, weight_hh, bias_ih, bias_hh):
    x = np.asarray(x, np.float32)
    Bx, T, Ix = x.shape
    st = _runner(T)
    cat = _prep_inputs(x, weight_ih, weight_hh, bias_ih, bias_hh, wm_key, T)
    res = np.empty((T, B, H), np.float32)
    res5 = res.reshape(T, NCORE, BC, 2, 128)
    if st["axon"]:
        zbuf = st["prev"]
        if zbuf is None:
            zbuf = st["zf"]()
        out_arrs = st["sharded"](*[cat[n] for n in st["in_names"]], zbuf)
        oarr = out_arrs[0]

        def fetch_unpack(shard):
            c = shard.index[0].start // T
            res5[:, c] = np.asarray(shard.data).astype(np.float32)

        with _cf.ThreadPoolExecutor(NCORE) as ex:
            list(ex.map(fetch_unpack, oarr.addressable_shards))
        st["prev"] = oarr  # donate this device buffer on the next call
        res *= 1.0 / OSCALE
        return res
    # native (non-axon) fallback: classic spmd runner
    from concourse.bass_utils import run_bass_kernel_spmd

    in_maps = []
    for c in range(NCORE):
        in_maps.append(
            {
                k: np.ascontiguousarray(
                    v.reshape((NCORE,) + (v.shape[0] // NCORE,) + v.shape[1:])[c]
                )
                for k, v in cat.items()
            }
        )
    rr = run_bass_kernel_spmd(st["nc"], in_maps, list(range(NCORE)))
    for c in range(NCORE):
        res5[:, c] = rr.results[c]["out"].astype(np.float32)
    res *= 1.0 / OSCALE
    return res


# revision 7
# speedup vs baseline: 1.1551x; 1.1551x over previous
"""KeyedGRU Trainium2 Bass kernel.

Strategy: data-parallel over batch B=64 across 8 cores (B=8 each), weights
replicated. Per core:
  Phase 0: 16-step key-gate GRU scan (KB=4) -> per-step gates g[16, H].
  Phase 1: 2048-step main GRU. The input-side matmul gi = x @ W_ih.T + bias
  is precomputed in 32-step chunks on the tensor engine (independent of h)
  and interleaved into the per-step idle windows; the sequential per-step
  work is gh = h @ W_hh.T (12 small matmuls, H-on-partitions layout),
  one sigmoid pass (r,i), the n-gate chain on DVE/ACT, and the lerp.
  tanh(z) is computed as 2*sigmoid(2z)-1 so the ACT engine never switches
  activation-table sets between Sigmoid and Tanh.

I/O path (the axon tunnel moves ~50-80 MB/s, so transfer bytes and host
reshapes dominate wall time):
  - x ships as f16 in its natural [BC, T, I] layout (host does a parallel
    dtype cast only, no transpose); the kernel XBAR-transposes the whole
    per-core x into a resident SBUF tile [128, 2, B*T] at startup, and the
    gi matmuls consume strided views of it in bf16 directly.
  - the output ships as int8 in host-natural [T, BC, H] layout: |h| <= 1
    always (h is a convex combination of tanh outputs starting from 0), so
    a fixed 1/127 scale with exact rint (magic-constant rounding) keeps the
    quantization error at 1/254 of absmax. The [128(h), t] -> [128(t), h]
    flip runs on the idle tensor engine (16 PE transposes per 128 steps).
  - the PJRT executable is compiled once per T and cached; the donated
    output buffer for call N+1 is call N's device-resident output, so no
    zero-buffer ships through the tunnel.
"""
import concurrent.futures as _cf

import ml_dtypes
import numpy as np

import concourse.bass as bass
import concourse.tile as tile
from concourse import mybir
from concourse.masks import make_identity

f32 = mybir.dt.float32
f16 = mybir.dt.float16
i8 = mybir.dt.int8
AF = mybir.ActivationFunctionType
ALU = mybir.AluOpType
F16 = np.float16

B, T_FULL, I, H = 64, 2048, 256, 256
KB, KL = 4, 16
NCORE = 8
BC = B // NCORE          # batch per core
M3 = 3 * H               # 768 gate outputs
CH = 32                  # gi chunk (steps)
OCH = 128                # output chunk (steps)
OSCALE = 127.0           # int8 output quantization scale
RMAGIC = 12582912.0      # 1.5 * 2^23: f32 add snaps mantissa to integer


def _fix_waits(nc, limit=1):
    """walrus TPB_CTRL encodes only one sync-wait; split extras onto nops."""
    for func in nc.m.functions:
        for bb in func.blocks:
            out = []
            for ins in bb.instructions:
                si = ins.sync_info
                if si and len(si.on_wait) > limit:
                    waits = list(si.on_wait)
                    for j, w in enumerate(waits[:-limit]):
                        nop = mybir.InstNoOp(name=f"{ins.name}-wfix{j}", ins=[], outs=[])
                        nop.engine = ins.engine
                        nop.sync_info = mybir.SyncInfo(on_wait=[w], on_update=[])
                        out.append(nop)
                    ins.sync_info = mybir.SyncInfo(
                        on_wait=list(waits[-limit:]), on_update=list(si.on_update)
                    )
                out.append(ins)
            bb.instructions = out


def _build(T):
    NCH = T // CH
    nc = bass.Bass("TRN2", num_devices=NCORE)
    x_in = nc.declare_dram_parameter("x", [BC, T, I], f16, isOutput=False)
    wih_d = nc.declare_dram_parameter("wih", [2, 128, M3], f16, isOutput=False)
    whh_d = nc.declare_dram_parameter("whh", [2, 128, M3], f16, isOutput=False)
    brow_d = nc.declare_dram_parameter("brow", [1, M3], f32, isOutput=False)
    bhn_d = nc.declare_dram_parameter("bhn", [2, 128, BC], f32, isOutput=False)
    wmk_d = nc.declare_dram_parameter("wmk", [2, 128, KL * KB], f32, isOutput=False)
    out_d = nc.declare_dram_parameter("out", [T, BC, 2, 128], i8, isOutput=True)

    with tile.TileContext(nc) as tc:
        with (
            tc.tile_pool(name="const", bufs=1) as const,
            tc.tile_pool(name="gips", bufs=2, space="PSUM") as gips,
            tc.tile_pool(name="ghps", bufs=1, space="PSUM") as ghps,
            tc.tile_pool(name="tpps", bufs=1, space="PSUM") as tpps,
            tc.tile_pool(name="gisb", bufs=2) as gisb,
            tc.tile_pool(name="outb", bufs=2) as outb,
            tc.tile_pool(name="qb", bufs=2) as qbp,
            tc.tile_pool(name="tmp", bufs=3) as tmp,
        ):
            # ---- constants ----
            wih_bf = const.tile([128, 2, M3], f16)
            whh_bf = const.tile([128, 2, M3], f16)
            for k in range(2):
                nc.sync.dma_start(out=wih_bf[:, k, :], in_=wih_d[k])
                nc.sync.dma_start(out=whh_bf[:, k, :], in_=whh_d[k])
            wih_sb = const.tile([128, 2, M3], f32)
            whh_sb = const.tile([128, 2, M3], f32)
            nc.vector.tensor_copy(wih_sb, wih_bf)
            nc.vector.tensor_copy(whh_sb, whh_bf)
            brow_sb = const.tile([1, M3], f32)
            nc.sync.dma_start(out=brow_sb, in_=brow_d[:, :])
            bhn_sb = const.tile([128, 2, BC], f32)
            for k in range(2):
                nc.sync.dma_start(out=bhn_sb[:, k, :], in_=bhn_d[k])
            kx_sb = const.tile([128, 2, KL * KB], f32)
            for k in range(2):
                nc.sync.dma_start(out=kx_sb[:, k, :], in_=wmk_d[k])
            ident = const.tile([128, 128], f32)
            make_identity(nc, ident)
            # whole per-core x, XBAR-transposed to put I%128 on partitions:
            # xall[p, k, b*T + t] = x[b, t, k*128+p]
            xall = const.tile([128, 2, BC * T], f16)
            x2d = x_in.rearrange("b t i -> (b t) i")
            for k in range(2):
                nc.sync.dma_start_transpose(
                    out=xall[:, k, :], in_=x2d[:, k * 128 : (k + 1) * 128]
                )
            xv = xall.rearrange("p k (b t) -> p k b t", b=BC)
            ones_sb = const.tile([1, CH * BC], f32)
            nc.vector.memset(ones_sb, 1.0)
            rbuf = const.tile([128, 2, KL, KB], f32)   # reset gates, key scan
            gr_sb = const.tile([128, 2, KL], f32)
            g_sb = const.tile([128, 2, KL], f32)
            h0 = const.tile([128, 2, BC], f32)
            nc.vector.memset(h0, 0.0)
            kgi_sb = const.tile([128, 6, KL * KB], f32)

            def mm(out_ap, lhsT, rhs, start, stop):
                nc.tensor.matmul(out_ap, lhsT, rhs, start=start, stop=stop)

            # ---- phase 0: key-gate scan (KB=4, KL=16) ----
            kgi_ps = gips.tile([128, 6, KL * KB], f32, tag="gi")
            for m in range(6):
                sl = slice(m * 128, (m + 1) * 128)
                mm(kgi_ps[:, m, :], wih_sb[:, 0, sl], kx_sb[:, 0, :], True, False)
                mm(kgi_ps[:, m, :], wih_sb[:, 1, sl], kx_sb[:, 1, :], False, False)
                mm(kgi_ps[:, m, :], brow_sb[:, sl], ones_sb[:, : KL * KB], False, True)
            nc.vector.tensor_copy(kgi_sb, kgi_ps)

            kh = tmp.tile([128, 2, KB], f32, tag="kh")
            nc.vector.memset(kh, 0.0)
            for t in range(KL):
                ksl = slice(t * KB, (t + 1) * KB)
                kgh = ghps.tile([128, 6, KB], f32, tag="gh")
                for m in range(6):
                    sl = slice(m * 128, (m + 1) * 128)
                    mm(kgh[:, m, :], whh_sb[:, 0, sl], kh[:, 0, :], True, False)
                    mm(kgh[:, m, :], whh_sb[:, 1, sl], kh[:, 1, :], False, True)
                sri = tmp.tile([128, 4, KB], f32, tag="sri")
                nc.vector.tensor_add(sri, kgh[:, 0:4, :], kgi_sb[:, 0:4, ksl])
                sig = tmp.tile([128, 4, KB], f32, tag="sig")
                nc.scalar.activation(sig, sri, AF.Sigmoid)
                nc.vector.tensor_copy(rbuf[:, :, t, :], sig[:, 0:2, :])
                t1 = tmp.tile([128, 2, KB], f32, tag="t1")
                nc.vector.tensor_add(t1, kgh[:, 4:6, :], bhn_sb[:, :, 0:KB])
                t2 = tmp.tile([128, 2, KB], f32, tag="t2")
                nc.vector.tensor_mul(t2, t1, sig[:, 0:2, :])
                t3 = tmp.tile([128, 2, KB], f32, tag="t3")
                nc.vector.tensor_add(t3, t2, kgi_sb[:, 4:6, ksl])
                ss = tmp.tile([128, 2, KB], f32, tag="ss")
                nc.scalar.activation(ss, t3, AF.Sigmoid, scale=2.0)
                nn = tmp.tile([128, 2, KB], f32, tag="nn")
                nc.vector.tensor_scalar(nn, ss, 2.0, -1.0, op0=ALU.mult, op1=ALU.add)
                dd = tmp.tile([128, 2, KB], f32, tag="dd")
                nc.vector.tensor_sub(dd, kh, nn)
                ee = tmp.tile([128, 2, KB], f32, tag="ee")
                nc.vector.tensor_mul(ee, dd, sig[:, 2:4, :])
                kh2 = tmp.tile([128, 2, KB], f32, tag="kh")
                nc.vector.tensor_add(kh2, ee, nn)
                kh = kh2
            nc.vector.tensor_reduce(gr_sb, rbuf, axis=mybir.AxisListType.X, op=ALU.add)
            nc.vector.tensor_scalar_mul(g_sb, gr_sb, 1.0 / KB)

            # ---- phase 1: main recurrence ----
            gi_ps_t, gi_sb_t = {}, {}
            pending = []  # deferred GI emission ops: ("mm", c, m, kk) | ("cp", c)

            def queue_gi(c):
                # gi chunk laid out [128, 6, BC, CH] (batch-major free dims,
                # matching xall's (b, t) order)
                gi_ps_t[c] = gips.tile([128, 6, BC, CH], f32, tag="gi", name=f"gi_ps{c}")
                gi_sb_t[c] = gisb.tile([128, 6, BC, CH], f32, tag="gis", name=f"gi_sb{c}")
                for m in range(6):
                    for kk in range(3):
                        pending.append(("mm", c, m, kk))
                pending.append(("cp", c))

            def emit_gi_op(op):
                _, c, m, kk = op if op[0] == "mm" else (None, op[1], None, None)
                if op[0] == "mm":
                    sl = slice(m * 128, (m + 1) * 128)
                    csl = slice(c * CH, (c + 1) * CH)
                    tgt = gi_ps_t[c][:, m, :, :]
                    if kk < 2:
                        mm(tgt, wih_bf[:, kk, sl], xv[:, kk, :, csl], kk == 0, False)
                    else:
                        mm(tgt, brow_sb[:, sl], ones_sb, False, True)
                else:
                    nc.vector.tensor_copy(gi_sb_t[c], gi_ps_t[c])

            # chunk 0 fully up-front; chunk 1 queued so it fills phase-0/early gaps
            queue_gi(0)
            while pending:
                emit_gi_op(pending.pop(0))
            if NCH > 1:
                queue_gi(1)

            hcur = lambda k: h0[:, k, :]      # per-Htile matmul rhs view
            hfull = h0[:, :, :]               # full [128, 2, BC] view for DVE
            ob = None
            for t in range(T):
                c, o = divmod(t, CH)
                ot = t % OCH
                if t % OCH == 0:
                    ob = outb.tile([128, 2, BC, OCH], f32, tag="ob")
                if t % CH == 0 and c + 2 < NCH:
                    queue_gi(c + 2)
                gh = ghps.tile([128, 6, BC], f32, tag="gh")
                for m in range(6):
                    sl = slice(m * 128, (m + 1) * 128)
                    mm(gh[:, m, :], whh_sb[:, 0, sl], hcur(0), True, False)
                    mm(gh[:, m, :], whh_sb[:, 1, sl], hcur(1), False, True)
                # fill PE idle windows with next chunk's gi work
                for _ in range(2):
                    if pending:
                        emit_gi_op(pending.pop(0))
                gsb = gi_sb_t[c]
                sri = tmp.tile([128, 4, BC], f32, tag="sri")
                nc.vector.tensor_add(sri, gh[:, 0:4, :], gsb[:, 0:4, :, o])
                sig = tmp.tile([128, 4, BC], f32, tag="sig")
                nc.scalar.activation(sig, sri, AF.Sigmoid)
                t1 = tmp.tile([128, 2, BC], f32, tag="t1")
                nc.vector.tensor_add(t1, gh[:, 4:6, :], bhn_sb)
                t2 = tmp.tile([128, 2, BC], f32, tag="t2")
                nc.vector.tensor_mul(t2, t1, sig[:, 0:2, :])
                t3 = tmp.tile([128, 2, BC], f32, tag="t3")
                nc.vector.tensor_add(t3, t2, gsb[:, 4:6, :, o])
                # nn = tanh(t3) without leaving the Sigmoid table set
                ss = tmp.tile([128, 2, BC], f32, tag="ss")
                nc.scalar.activation(ss, t3, AF.Sigmoid, scale=2.0)
                nn = tmp.tile([128, 2, BC], f32, tag="nn")
                nc.vector.tensor_scalar(nn, ss, 2.0, -1.0, op0=ALU.mult, op1=ALU.add)
                dd = tmp.tile([128, 2, BC], f32, tag="dd")
                nc.vector.tensor_sub(dd, hfull, nn)
                ee = tmp.tile([128, 2, BC], f32, tag="ee")
                nc.vector.tensor_mul(ee, dd, sig[:, 2:4, :])
                nc.vector.tensor_add(ob[:, :, :, ot], ee, nn)
                if t < KL:
                    hg = tmp.tile([128, 2, BC], f32, tag="hg")
                    for k in range(2):
                        nc.vector.tensor_scalar(
                            hg[:, k, :], ob[:, k, :, ot], g_sb[:, k, t : t + 1],
                            None, op0=ALU.mult,
                        )
                    hcur = (lambda hg_: lambda k: hg_[:, k, :])(hg)
                    hfull = hg[:, :, :]
                else:
                    hcur = (lambda ob_, ot_: lambda k: ob_[:, k, :, ot_])(ob, ot)
                    hfull = ob[:, :, :, ot]
                if ot == OCH - 1:
                    # quantize to int8 in host-natural [t, b, h] layout:
                    # rint via magic constant, PE transpose of each [128h, 128t]
                    # block, then exact integer subtract + int8 cast on DVE.
                    rb = qbp.tile([128, 2, BC, OCH], f32, tag="rb")
                    nc.vector.tensor_scalar(
                        rb, ob, OSCALE, RMAGIC, op0=ALU.mult, op1=ALU.add
                    )
                    obt = qbp.tile([128, BC, 2, 128], i8, tag="obt")
                    for k in range(2):
                        for b in range(BC):
                            tp = tpps.tile([128, 128], f32, tag="tp")
                            nc.tensor.transpose(tp, rb[:, k, b, :], ident)
                            nc.vector.tensor_scalar(
                                obt[:, b, k, :], tp, -RMAGIC, None, op0=ALU.add
                            )
                    nc.sync.dma_start(
                        out=out_d[t - OCH + 1 : t + 1, :, :, :], in_=obt
                    )

    _fix_waits(nc)
    return nc


_RUN = {}


def _runner(T):
    st = _RUN.get(T)
    if st is not None:
        return st
    nc = _build(T)
    st = {"nc": nc, "prev": None}
    from concourse._compat import axon_active

    if axon_active():
        import jax
        import jax.numpy as jnp
        from jax.experimental.shard_map import shard_map
        from jax.sharding import Mesh, NamedSharding, PartitionSpec

        from concourse.bass2jax import (
            _bass_exec_p,
            install_neuronx_cc_hook,
            partition_id_tensor,
        )

        install_neuronx_cc_hook()
        pname = nc.partition_id_tensor.name if nc.partition_id_tensor else None
        in_names, out_names, out_avals = [], [], []
        for alloc in nc.m.functions[0].allocations:
            if not isinstance(alloc, mybir.MemoryLocationSet):
                continue
            name = alloc.memorylocations[0].name
            if alloc.kind == "ExternalInput":
                if name != pname:
                    in_names.append(name)
            elif alloc.kind == "ExternalOutput":
                out_names.append(name)
                out_avals.append(
                    jax.core.ShapedArray(
                        tuple(alloc.tensor_shape), mybir.dt.np(alloc.dtype)
                    )
                )
        n_params = len(in_names)
        n_outs = len(out_avals)
        in_names_all = in_names + out_names + ([pname] if pname else [])
        donate = tuple(range(n_params, n_params + n_outs))

        def _body(*args):
            ops = list(args)
            if pname:
                ops.append(partition_id_tensor())
            return tuple(
                _bass_exec_p.bind(
                    *ops,
                    out_avals=tuple(out_avals),
                    in_names=tuple(in_names_all),
                    out_names=tuple(out_names),
                    lowering_input_output_aliases=(),
                    sim_require_finite=True,
                    sim_require_nnan=True,
                    nc=nc,
                )
            )

        mesh = Mesh(np.asarray(jax.devices()[:NCORE]), ("core",))
        sh = NamedSharding(mesh, PartitionSpec("core"))
        in_specs = (PartitionSpec("core"),) * (n_params + n_outs)
        out_specs = (PartitionSpec("core"),) * n_outs
        sharded = jax.jit(
            shard_map(
                _body, mesh=mesh, in_specs=in_specs, out_specs=out_specs,
                check_rep=False,
            ),
            donate_argnums=donate,
            keep_unused=True,
        )
        oshape = out_avals[0].shape
        zf = jax.jit(
            lambda: jnp.zeros((NCORE * oshape[0],) + oshape[1:], jnp.int8),
            out_shardings=sh,
        )
        st.update(
            axon=True, sharded=sharded, zf=zf, in_names=in_names,
            oshape=oshape, jax=jax,
        )
    else:
        st.update(axon=False)
    _RUN[T] = st
    return st


def _prep_inputs(x, weight_ih, weight_hh, bias_ih, bias_hh, wm_key, T):
    """Host-side packing: per-input arrays concatenated over cores on axis 0.
    x needs only a parallel f32 -> bf16 cast (layout stays [B, T, I])."""
    wih = np.ascontiguousarray(weight_ih.T.reshape(2, 128, M3)).astype(F16)
    whh = np.ascontiguousarray(weight_hh.T.reshape(2, 128, M3)).astype(F16)
    brow = (
        np.asarray(bias_ih, np.float32)
        + np.concatenate(
            [np.asarray(bias_hh[: 2 * H], np.float32), np.zeros(H, np.float32)]
        )
    ).reshape(1, M3)
    bhn = np.ascontiguousarray(
        np.tile(np.asarray(bias_hh[2 * H :], np.float32).reshape(2, 128, 1), (1, 1, BC))
    )
    wmk = np.ascontiguousarray(
        wm_key.transpose(2, 1, 0).reshape(2, 128, KL * KB), np.float32
    )
    try:  # torch's vectorized cast is ~3x numpy's on one core
        import torch

        xbuf = torch.from_numpy(x).to(torch.float16).numpy()
    except ImportError:
        xbuf = x.astype(F16)
    reps = {"wih": wih, "whh": whh, "brow": brow, "bhn": bhn, "wmk": wmk}
    cat = {k: np.concatenate([v] * NCORE, axis=0) for k, v in reps.items()}
    cat["x"] = xbuf
    return cat


def kernel(x, wm_key, weight_ih, weight_hh, bias_ih, bias_hh):
    x = np.asarray(x, np.float32)
    Bx, T, Ix = x.shape
    st = _runner(T)
    cat = _prep_inputs(x, weight_ih, weight_hh, bias_ih, bias_hh, wm_key, T)
    res = np.empty((T, B, H), np.float32)
    res5 = res.reshape(T, NCORE, BC, 2, 128)
    if st["axon"]:
        zbuf = st["prev"]
        if zbuf is None:
            zbuf = st["zf"]()
        out_arrs = st["sharded"](*[cat[n] for n in st["in_names"]], zbuf)
        oarr = out_arrs[0]

        def fetch_unpack(shard):
            c = shard.index[0].start // T
            res5[:, c] = np.asarray(shard.data).astype(np.float32)

        with _cf.ThreadPoolExecutor(NCORE) as ex:
            list(ex.map(fetch_unpack, oarr.addressable_shards))
        st["prev"] = oarr  # donate this device buffer on the next call
        res *= 1.0 / OSCALE
        return res
    # native (non-axon) fallback: classic spmd runner
    from concourse.bass_utils import run_bass_kernel_spmd

    in_maps = []
    for c in range(NCORE):
        in_maps.append(
            {
                k: np.ascontiguousarray(
                    v.reshape((NCORE,) + (v.shape[0] // NCORE,) + v.shape[1:])[c]
                )
                for k, v in cat.items()
            }
        )
    rr = run_bass_kernel_spmd(st["nc"], in_maps, list(range(NCORE)))
    for c in range(NCORE):
        res5[:, c] = rr.results[c]["out"].astype(np.float32)
    res *= 1.0 / OSCALE
    return res
